# revision 58
# baseline (speedup 1.0000x reference)
"""Trainium2 Bass kernel for nn_AttentionNetwork (gnn_message_passing).

Computes, for f_meta [N, D] and W [2D, M] (N=4096, D=128, M=4):
    a = f_meta @ W[:D]            # [N, M]
    b = f_meta @ W[D:]            # [N, M]
    s = exp(relu(a[:,None,:] + b[None,:,:]))     # [N, N, M]
    out = s / sum(s, axis=-1, keepdims=True)

Key identity used on device:  exp(relu(x)) = max(exp(x), 1), and
exp(a+b) = exp(a)*exp(b).  So with ea = exp(a), eb = exp(b):
    t[i,j,m] = ea[i,m] * eb[j,m]
    s = max(t, 1) = relu(t - 1) + 1
    out = (r + 1) / (sum_m r + 4)     where r = relu(t - 1)

Sharding: row-parallel over source nodes i across 8 cores; each core
computes a [512, 4096, 4] slab. The N^2*M product grid is produced on
the TensorEngine as bf16 hi/lo contract-12 matmuls against an
interleaved selector matrix S3[4k+m, 4j+m'] = delta(m,m') *
(ebh, ebh, ebl)[k][j, m], which yields eah*ebh + eal*ebh + eah*ebl
(~4e-6 relative error vs the fp32 product) with the output laid out
[i, (j, m)] with m innermost, so all 256MB of HBM writes are fully
contiguous. Normalization: sum over m via pairwise adds (GpSimd+DVE),
1/(d+4) via exp(-ln((d+4)*2^-64) - 64*ln2) on ScalarE (ln input
rescaled into its valid range), and a fused (r+1)*recip
scalar_tensor_tensor on VectorE, in place, followed by one 4MB
contiguous DMA store per half i-block.
"""

import os
import sys

import numpy as np

for _p in ("/root/.axon_site/_ro/trn_rl_repo", "/opt/trn_rl_repo"):
    if os.path.isdir(_p) and _p not in sys.path:
        sys.path.append(_p)

import concourse.bass as bass
import concourse.mybir as mybir
import concourse.tile as tile
from concourse import bacc
from concourse.bass_utils import run_bass_kernel_spmd

N = 4096          # number of nodes (j dimension)
D = 128           # feature dim
M = 4             # num meta paths
NCORES = 8
R = N // NCORES   # rows (i) per core = 512
F = N * M         # interleaved free size = 16384
FP = mybir.dt.float32
BF = mybir.dt.bfloat16

_CACHE = {}

# Force all activations (Relu/Exp/Ln) onto the single table set that
# contains all three ("natural_log_exp_and_others"); the default greedy
# set picker alternates exp_and_others <-> natural_log per half-block,
# paying a ~2.7us ACT table reload each time.
_ONE_SET = "natural_log_exp_and_others"
_orig_get_tables = None


def _patched_get_tables(arch):
    tabs = _orig_get_tables(arch)
    _keep = {
        mybir.ActivationFunctionType.Relu,
        mybir.ActivationFunctionType.Exp,
        mybir.ActivationFunctionType.Ln,
    }
    out = {}
    for name, fns in tabs.items():
        if name == _ONE_SET:
            out[name] = fns
        else:
            out[name] = {f for f in fns if f not in _keep}
    return out


def _install_table_patch():
    global _orig_get_tables
    from concourse import bacc as _bacc, hw_specs as _hw
    if _orig_get_tables is None:
        _orig_get_tables = _hw.get_activation_tables
        _hw.get_activation_tables = _patched_get_tables
        _bacc.get_activation_tables = _patched_get_tables


def _build_nc():
    _install_table_patch()
    nc = bacc.Bacc(
        "TRN2",
        target_bir_lowering=False,
        debug=False,
        enable_asserts=False,
        num_devices=NCORES,
    )

    f_full = nc.dram_tensor("fT_full", [D, N], FP, kind="ExternalInput").ap()
    f_mine = nc.dram_tensor("fT_mine", [D, R], FP, kind="ExternalInput").ap()
    s_mask = nc.dram_tensor("s_mask", [3 * M, F], BF, kind="ExternalInput").ap()
    out = nc.dram_tensor("out", [R, F], FP, kind="ExternalOutput").ap()

    w_dram = nc.dram_tensor("w", [2 * D, M], FP, kind="ExternalInput").ap()

    with tile.TileContext(nc) as tc:
        _emit(tc, out, f_full, f_mine, s_mask, w_dram)

    nc.compile()
    return nc


def _emit(tc, out, f_full, f_mine, s_mask, w_dram):
    nc = tc.nc
    AF = mybir.ActivationFunctionType
    OP = mybir.AluOpType

    n_fb = N // D            # 32 blocks of f_full
    n_mb = R // D            # 4 blocks of f_mine

    from contextlib import ExitStack
    ctx = ExitStack()
    # ---- persistent tiles -------------------------------------------------
    const_pool = ctx.enter_context(tc.tile_pool(name="const", bufs=1))
    bias_m1 = const_pool.tile([128, 1], FP)   # -1.0 for relu(t-1)
    nc.gpsimd.memset(bias_m1[:], -1.0)
    # ln on ACT only accepts |x| <= 2^64 but d+4 reaches ~4e36, so compute
    # g = ln((d+4) * 2^-64) and 1/(d+4) = exp(-g - 64*ln2).
    LNSCALE = 2.0 ** -64
    bias_4c = const_pool.tile([128, 1], FP)   # 4 * 2^-64 for ln((d+4)*c)
    nc.gpsimd.memset(bias_4c[:], 4.0 * LNSCALE)
    bias_mln = const_pool.tile([128, 1], FP)  # -64*ln2 for exp(-g - 64 ln2)
    nc.gpsimd.memset(bias_mln[:], -44.3614195558365)
    # S3[4k+m, 4j+m'] = delta(m,m') * (ebh, ebh, ebl)[k][j, m]  (bf16)
    # eaHL[4k+m, i]    = (eah, eal, eah)[k][i, m]                (bf16)
    # so eaHL.T @ S3 = eah*ebh + eal*ebh + eah*ebl ~ ea*eb (err ~4e-6)
    S3 = const_pool.tile([3 * M, F], BF)
    eaHL = const_pool.tile([3 * M, R], BF)

    # ---- setup (transient tiles freed before steady state) ----------------
    with tc.tile_pool(name="setup_const", bufs=1) as scp, \
         tc.tile_pool(name="setup_ps2", bufs=5, space="PSUM") as psb:
        # input loads go through the ACT-issued HWDGE ring so they don't
        # queue behind anything on the SP ring; small/urgent ones first,
        # and the big fT load is chunked so dependents start early.
        wa = scp.tile([D, M], FP)       # W[:D]
        wb = scp.tile([D, M], FP)       # W[D:]
        fmT = scp.tile([D, R], FP)
        nc.sync.dma_start(fmT[:], f_mine)
        nc.sync.dma_start(wa[:], w_dram[0:D, :])
        nc.sync.dma_start(wb[:], w_dram[D:2 * D, :])

        ffT = scp.tile([D, N], FP)
        mask12 = scp.tile([3 * M, F], BF)
        for g in range(N // 1024):
            nc.scalar.dma_start(ffT[:, g * 1024:(g + 1) * 1024],
                                f_full[:, g * 1024:(g + 1) * 1024])
            # mask12[4k+m, 4j+m'] = (m == m')
            nc.sync.dma_start(mask12[:, g * (F // 4):(g + 1) * (F // 4)],
                              s_mask[:, g * (F // 4):(g + 1) * (F // 4)])

        ebT = scp.tile([M, N], FP)      # exp(b).T
        eaT = scp.tile([M, R], FP)      # exp(a_mine).T
        ebHL = scp.tile([3 * M, N], BF)     # rows: ebh(4), ebh(4), ebl(4)
        ebl = scp.tile([M, N], BF)
        eal = scp.tile([M, R], BF)

        def expT_slice(fsrc, w_t, dstT, j0, jn):
            """dstT[:, j0:j0+jn] = exp(w_t.T @ fsrc[:, j0:j0+jn])"""
            for q0 in range(j0, j0 + jn, 512):
                qn = min(512, j0 + jn - q0)
                bp = psb.tile([M, 512], FP, tag="bT")
                nc.tensor.matmul(bp[:, 0:qn], w_t[:], fsrc[:, q0:q0 + qn],
                                 start=True, stop=True)
                nc.scalar.activation(dstT[:, q0:q0 + qn], bp[:, 0:qn], AF.Exp)

        # a-side first (tiny) so steady state can begin as soon as the
        # first S3 slice lands.
        expT_slice(fmT, wa, eaT, 0, R)
        nc.scalar.copy(eaHL[0:M, :], eaT[:])          # eah (bf16 cast)
        nc.vector.tensor_tensor(eal[:], eaT[:], eaHL[0:M, :], op=OP.subtract)
        nc.sync.dma_start(eaHL[M:2 * M, :], eal[:])
        nc.sync.dma_start(eaHL[2 * M:3 * M, :], eaHL[0:M, :])

        # b-side: per 512-j group, do the whole chain slice-locally so
        # everything pipelines.
        for g in range(N // 512):
            j0 = g * 512
            sl = slice(j0, j0 + 512)
            expT_slice(ffT, wb, ebT, j0, 512)
            nc.scalar.copy(ebHL[0:M, sl], ebT[:, sl])
            nc.vector.tensor_tensor(ebl[:, sl], ebT[:, sl], ebHL[0:M, sl],
                                    op=OP.subtract)
            nc.sync.dma_start(ebHL[M:2 * M, sl], ebHL[0:M, sl])
            nc.sync.dma_start(ebHL[2 * M:3 * M, sl], ebl[:, sl])
            # S3 slice = broadcast4(ebHL slice) * mask12 slice
            for q in range(2):
                c0 = j0 * M + q * 1024
                jj = c0 // M
                jn = 1024 // M
                eng = nc.gpsimd if q == 1 else nc.vector
                eng.tensor_tensor(
                    S3[:, c0:c0 + 1024].rearrange("p (j m) -> p j m", m=M),
                    ebHL[:, jj:jj + jn].broadcast_to((3 * M, jn, M)),
                    mask12[:, c0:c0 + 1024].rearrange("p (j m) -> p j m", m=M),
                    op=OP.mult,
                )

    # ---- steady state -----------------------------------------------------
    # per i-block (128 rows), per half (8192 interleaved cols = 2048 j):
    #   PE:   t[128, 2048] = eaT_slice.T @ S_slice   (4 matmuls of 512)
    #   ACT:  r = relu(t - 1)                        (PSUM -> SBUF)
    #   GP:   rp = pairwise sum of r                 (stride-2 adds)
    #   DVE:  d  = pairwise sum of rp  -> sum_m r
    #   ACT:  g = ln(d + 4) ; eg = exp(-g)           (= 1/(d+4))
    #   DVE:  out = (r + 1) * eg[bcast4]             (in-place over r)
    #   DMA:  4MB contiguous store
    HALF = F // 2            # 8192
    JH = HALF // M           # 2048 j per half
    CH = 2048                # interleaved cols per chunk (one PSUM tensor)
    n_ch = HALF // CH        # 4 chunks per half

    with tc.tile_pool(name="ps_t", bufs=2, space="PSUM") as ps_t, \
         tc.tile_pool(name="big", bufs=4) as bigp, \
         tc.tile_pool(name="rp", bufs=4) as rpp, \
         tc.tile_pool(name="dsum", bufs=3) as dsp:
        JC = CH // M          # 512 j per chunk
        for ib in range(R // 128):
            ea_sl = eaHL[:, ib * 128:(ib + 1) * 128]
            for h in range(2):
                f0 = h * HALF
                big = bigp.tile([128, HALF], FP, tag="big")
                dsum = dsp.tile([128, JH], FP, tag="dsum")
                for c in range(n_ch):
                    tp = ps_t.tile([128, CH], FP, tag="tp")
                    for q in range(CH // 512):
                        co = c * CH + q * 512
                        nc.tensor.matmul(
                            tp[:, q * 512:(q + 1) * 512],
                            ea_sl, S3[:, f0 + co:f0 + co + 512],
                            start=True, stop=True,
                        )
                    rr = big[:, c * CH:(c + 1) * CH]
                    # r = relu(t - 1)
                    nc.scalar.activation(rr, tp[:], AF.Relu, bias=bias_m1[:])
                    # pairwise adds: rp[k] = r[2k] + r[2k+1]
                    rp = rpp.tile([128, CH // 2], FP, tag="rp")
                    p1eng = nc.vector if c == 3 else nc.gpsimd
                    p1eng.tensor_tensor(
                        rp[:], rr[:, 0::2], rr[:, 1::2], op=OP.add,
                    )
                    ds = dsum[:, c * JC:(c + 1) * JC]
                    nc.gpsimd.tensor_tensor(
                        ds, rp[:, 0::2], rp[:, 1::2], op=OP.add,
                    )
                    if c % 2 == 1:
                        # 1/(d+4) via exp(-ln((d+4)c) - 64 ln2) on ACT
                        ds2 = dsum[:, (c - 1) * JC:(c + 1) * JC]
                        nc.scalar.activation(ds2, ds2, AF.Ln,
                                             bias=bias_4c[:], scale=LNSCALE)
                        nc.scalar.activation(ds2, ds2, AF.Exp,
                                             bias=bias_mln[:], scale=-1.0)
                        for cc in (c - 1, c):
                            rrc = big[:, cc * CH:(cc + 1) * CH]
                            rr3 = rrc.rearrange("p (j m) -> p j m", m=M)
                            dsc = dsum[:, cc * JC:(cc + 1) * JC]
                            nc.vector.scalar_tensor_tensor(
                                rr3, rr3, 1.0,
                                dsc.broadcast_to((128, JC, M)),
                                op0=OP.add, op1=OP.mult,
                            )
                if (ib, h) == (R // 128 - 1, 1):
                    for piece in range(2):
                        p0 = piece * (HALF // 2)
                        nc.sync.dma_start(
                            out[ib * 128:(ib + 1) * 128,
                                f0 + p0:f0 + p0 + HALF // 2],
                            big[:, p0:p0 + HALF // 2],
                        )
                else:
                    nc.sync.dma_start(
                        out[ib * 128:(ib + 1) * 128, f0:f0 + HALF], big[:],
                    )
    ctx.close()


def kernel(f_meta: np.ndarray, W: np.ndarray) -> np.ndarray:
    f_meta = np.ascontiguousarray(f_meta, dtype=np.float32)
    W = np.ascontiguousarray(W, dtype=np.float32)
    assert f_meta.shape == (N, D) and W.shape == (2 * D, M)

    if "nc" not in _CACHE:
        _CACHE["nc"] = _build_nc()
    nc = _CACHE["nc"]

    import ml_dtypes
    mask = np.zeros((3 * M, F), dtype=ml_dtypes.bfloat16)
    for k in range(3):
        for m in range(M):
            mask[k * M + m, m::M] = 1.0
    fT = np.ascontiguousarray(f_meta.T)
    in_maps = [
        {
            "fT_full": fT,
            "fT_mine": np.ascontiguousarray(fT[:, c * R:(c + 1) * R]),
            "s_mask": mask,
            "w": W,
        }
        for c in range(NCORES)
    ]
    res = run_bass_kernel_spmd(nc, in_maps, core_ids=list(range(NCORES)))
    slabs = [res.results[c]["out"] for c in range(NCORES)]
    return np.concatenate(slabs, axis=0).reshape(N, N, M)


if __name__ == "__main__":
    f = np.random.randn(N, D).astype(np.float32)
    w = np.random.randn(2 * D, M).astype(np.float32)
    o = kernel(f, w)
    print(o.shape, o.dtype, o[0, 0], o.sum(axis=-1).mean())


# revision 60
# speedup vs baseline: 1.0131x; 1.0131x over previous
"""Trainium2 Bass kernel for nn_AttentionNetwork (gnn_message_passing).

Computes, for f_meta [N, D] and W [2D, M] (N=4096, D=128, M=4):
    a = f_meta @ W[:D]            # [N, M]
    b = f_meta @ W[D:]            # [N, M]
    s = exp(relu(a[:,None,:] + b[None,:,:]))     # [N, N, M]
    out = s / sum(s, axis=-1, keepdims=True)

Key identity used on device:  exp(relu(x)) = max(exp(x), 1), and
exp(a+b) = exp(a)*exp(b).  So with ea = exp(a), eb = exp(b):
    t[i,j,m] = ea[i,m] * eb[j,m]
    s = max(t, 1) = relu(t - 1) + 1
    out = (r + 1) / (sum_m r + 4)     where r = relu(t - 1)

Sharding: row-parallel over source nodes i across 8 cores; each core
computes a [512, 4096, 4] slab. The N^2*M product grid is produced on
the TensorEngine as bf16 hi/lo contract-12 matmuls against an
interleaved selector matrix S3[4k+m, 4j+m'] = delta(m,m') *
(ebh, ebh, ebl)[k][j, m], which yields eah*ebh + eal*ebh + eah*ebl
(~4e-6 relative error vs the fp32 product) with the output laid out
[i, (j, m)] with m innermost, so all 256MB of HBM writes are fully
contiguous. Normalization: sum over m via pairwise adds (GpSimd+DVE),
1/(d+4) via exp(-ln((d+4)*2^-64) - 64*ln2) on ScalarE (ln input
rescaled into its valid range), and a fused (r+1)*recip
scalar_tensor_tensor on VectorE, in place, followed by one 4MB
contiguous DMA store per half i-block.
"""

import os
import sys

import numpy as np

for _p in ("/root/.axon_site/_ro/trn_rl_repo", "/opt/trn_rl_repo"):
    if os.path.isdir(_p) and _p not in sys.path:
        sys.path.append(_p)

import concourse.bass as bass
import concourse.mybir as mybir
import concourse.tile as tile
from concourse import bacc
from concourse.bass_utils import run_bass_kernel_spmd

N = 4096          # number of nodes (j dimension)
D = 128           # feature dim
M = 4             # num meta paths
NCORES = 8
R = N // NCORES   # rows (i) per core = 512
F = N * M         # interleaved free size = 16384
FP = mybir.dt.float32
BF = mybir.dt.bfloat16

_CACHE = {}

# Force all activations (Relu/Exp/Ln) onto the single table set that
# contains all three ("natural_log_exp_and_others"); the default greedy
# set picker alternates exp_and_others <-> natural_log per half-block,
# paying a ~2.7us ACT table reload each time.
_ONE_SET = "natural_log_exp_and_others"
_orig_get_tables = None


def _patched_get_tables(arch):
    tabs = _orig_get_tables(arch)
    _keep = {
        mybir.ActivationFunctionType.Relu,
        mybir.ActivationFunctionType.Exp,
        mybir.ActivationFunctionType.Ln,
    }
    out = {}
    for name, fns in tabs.items():
        if name == _ONE_SET:
            out[name] = fns
        else:
            out[name] = {f for f in fns if f not in _keep}
    return out


def _install_table_patch():
    global _orig_get_tables
    from concourse import bacc as _bacc, hw_specs as _hw
    if _orig_get_tables is None:
        _orig_get_tables = _hw.get_activation_tables
        _hw.get_activation_tables = _patched_get_tables
        _bacc.get_activation_tables = _patched_get_tables


def _build_nc():
    _install_table_patch()
    nc = bacc.Bacc(
        "TRN2",
        target_bir_lowering=False,
        debug=False,
        enable_asserts=False,
        num_devices=NCORES,
    )

    f_full = nc.dram_tensor("fT_full", [D, N], FP, kind="ExternalInput").ap()
    f_mine = nc.dram_tensor("fT_mine", [D, R], FP, kind="ExternalInput").ap()
    s_mask = nc.dram_tensor("s_mask", [3 * M, F], BF, kind="ExternalInput").ap()
    out = nc.dram_tensor("out", [R, F], FP, kind="ExternalOutput").ap()

    w_dram = nc.dram_tensor("w", [2 * D, M], FP, kind="ExternalInput").ap()

    with tile.TileContext(nc) as tc:
        _emit(tc, out, f_full, f_mine, s_mask, w_dram)

    nc.compile()
    return nc


def _emit(tc, out, f_full, f_mine, s_mask, w_dram):
    nc = tc.nc
    AF = mybir.ActivationFunctionType
    OP = mybir.AluOpType

    n_fb = N // D            # 32 blocks of f_full
    n_mb = R // D            # 4 blocks of f_mine

    from contextlib import ExitStack
    ctx = ExitStack()
    # ---- persistent tiles -------------------------------------------------
    const_pool = ctx.enter_context(tc.tile_pool(name="const", bufs=1))
    bias_m1 = const_pool.tile([128, 1], FP)   # -1.0 for relu(t-1)
    nc.gpsimd.memset(bias_m1[:], -1.0)
    # ln on ACT only accepts |x| <= 2^64 but d+4 reaches ~4e36, so compute
    # g = ln((d+4) * 2^-64) and 1/(d+4) = exp(-g - 64*ln2).
    LNSCALE = 2.0 ** -64
    bias_4c = const_pool.tile([128, 1], FP)   # 4 * 2^-64 for ln((d+4)*c)
    nc.gpsimd.memset(bias_4c[:], 4.0 * LNSCALE)
    bias_mln = const_pool.tile([128, 1], FP)  # -64*ln2 for exp(-g - 64 ln2)
    nc.gpsimd.memset(bias_mln[:], -44.3614195558365)
    # S3[4k+m, 4j+m'] = delta(m,m') * (ebh, ebh, ebl)[k][j, m]  (bf16)
    # eaHL[4k+m, i]    = (eah, eal, eah)[k][i, m]                (bf16)
    # so eaHL.T @ S3 = eah*ebh + eal*ebh + eah*ebl ~ ea*eb (err ~4e-6)
    S3 = const_pool.tile([3 * M, F], BF)
    eaHL = const_pool.tile([3 * M, R], BF)

    # ---- setup (transient tiles freed before steady state) ----------------
    with tc.tile_pool(name="setup_const", bufs=1) as scp, \
         tc.tile_pool(name="setup_ps2", bufs=5, space="PSUM") as psb:
        # input loads go through the ACT-issued HWDGE ring so they don't
        # queue behind anything on the SP ring; small/urgent ones first,
        # and the big fT load is chunked so dependents start early.
        wa = scp.tile([D, M], FP)       # W[:D]
        wb = scp.tile([D, M], FP)       # W[D:]
        fmT = scp.tile([D, R], FP)
        nc.sync.dma_start(fmT[:], f_mine)
        nc.sync.dma_start(wa[:], w_dram[0:D, :])
        nc.sync.dma_start(wb[:], w_dram[D:2 * D, :])

        ffT = scp.tile([D, N], FP)
        mask12 = scp.tile([3 * M, F], BF)
        for g in range(N // 1024):
            nc.scalar.dma_start(ffT[:, g * 1024:(g + 1) * 1024],
                                f_full[:, g * 1024:(g + 1) * 1024])
            # mask12[4k+m, 4j+m'] = (m == m')
            nc.sync.dma_start(mask12[:, g * (F // 4):(g + 1) * (F // 4)],
                              s_mask[:, g * (F // 4):(g + 1) * (F // 4)])

        ebT = scp.tile([M, N], FP)      # exp(b).T
        eaT = scp.tile([M, R], FP)      # exp(a_mine).T
        ebHL = scp.tile([3 * M, N], BF)     # rows: ebh(4), ebh(4), ebl(4)
        ebl = scp.tile([M, N], BF)
        eal = scp.tile([M, R], BF)

        def expT_slice(fsrc, w_t, dstT, j0, jn):
            """dstT[:, j0:j0+jn] = exp(w_t.T @ fsrc[:, j0:j0+jn])"""
            for q0 in range(j0, j0 + jn, 512):
                qn = min(512, j0 + jn - q0)
                bp = psb.tile([M, 512], FP, tag="bT")
                nc.tensor.matmul(bp[:, 0:qn], w_t[:], fsrc[:, q0:q0 + qn],
                                 start=True, stop=True)
                nc.scalar.activation(dstT[:, q0:q0 + qn], bp[:, 0:qn], AF.Exp)

        # a-side first (tiny) so steady state can begin as soon as the
        # first S3 slice lands.
        expT_slice(fmT, wa, eaT, 0, R)
        nc.vector.tensor_copy(eaHL[0:M, :], eaT[:])   # eah (bf16 cast)
        nc.vector.tensor_tensor(eal[:], eaT[:], eaHL[0:M, :], op=OP.subtract)
        nc.sync.dma_start(eaHL[M:2 * M, :], eal[:])
        nc.sync.dma_start(eaHL[2 * M:3 * M, :], eaHL[0:M, :])

        # b-side: per 512-j group, do the whole chain slice-locally so
        # everything pipelines.
        for g in range(N // 512):
            j0 = g * 512
            sl = slice(j0, j0 + 512)
            expT_slice(ffT, wb, ebT, j0, 512)
            nc.vector.tensor_copy(ebHL[0:M, sl], ebT[:, sl])
            nc.vector.tensor_tensor(ebl[:, sl], ebT[:, sl], ebHL[0:M, sl],
                                    op=OP.subtract)
            nc.sync.dma_start(ebHL[M:2 * M, sl], ebHL[0:M, sl])
            nc.sync.dma_start(ebHL[2 * M:3 * M, sl], ebl[:, sl])
            # S3 slice = broadcast4(ebHL slice) * mask12 slice
            for q in range(2):
                c0 = j0 * M + q * 1024
                jj = c0 // M
                jn = 1024 // M
                eng = nc.gpsimd if q == 1 else nc.vector
                eng.tensor_tensor(
                    S3[:, c0:c0 + 1024].rearrange("p (j m) -> p j m", m=M),
                    ebHL[:, jj:jj + jn].broadcast_to((3 * M, jn, M)),
                    mask12[:, c0:c0 + 1024].rearrange("p (j m) -> p j m", m=M),
                    op=OP.mult,
                )

    # ---- steady state -----------------------------------------------------
    # per i-block (128 rows), per half (8192 interleaved cols = 2048 j):
    #   PE:   t[128, 2048] = eaT_slice.T @ S_slice   (4 matmuls of 512)
    #   ACT:  r = relu(t - 1)                        (PSUM -> SBUF)
    #   GP:   rp = pairwise sum of r                 (stride-2 adds)
    #   DVE:  d  = pairwise sum of rp  -> sum_m r
    #   ACT:  g = ln(d + 4) ; eg = exp(-g)           (= 1/(d+4))
    #   DVE:  out = (r + 1) * eg[bcast4]             (in-place over r)
    #   DMA:  4MB contiguous store
    HALF = F // 2            # 8192
    JH = HALF // M           # 2048 j per half
    CH = 2048                # interleaved cols per chunk (one PSUM tensor)
    n_ch = HALF // CH        # 4 chunks per half

    with tc.tile_pool(name="ps_t", bufs=2, space="PSUM") as ps_t, \
         tc.tile_pool(name="big", bufs=4) as bigp, \
         tc.tile_pool(name="rp", bufs=4) as rpp, \
         tc.tile_pool(name="dsum", bufs=3) as dsp:
        JC = CH // M          # 512 j per chunk
        for ib in range(R // 128):
            ea_sl = eaHL[:, ib * 128:(ib + 1) * 128]
            for h in range(2):
                f0 = h * HALF
                big = bigp.tile([128, HALF], FP, tag="big")
                dsum = dsp.tile([128, JH], FP, tag="dsum")
                for c in range(n_ch):
                    tp = ps_t.tile([128, CH], FP, tag="tp")
                    for q in range(CH // 512):
                        co = c * CH + q * 512
                        nc.tensor.matmul(
                            tp[:, q * 512:(q + 1) * 512],
                            ea_sl, S3[:, f0 + co:f0 + co + 512],
                            start=True, stop=True,
                        )
                    rr = big[:, c * CH:(c + 1) * CH]
                    # r = relu(t - 1)
                    nc.scalar.activation(rr, tp[:], AF.Relu, bias=bias_m1[:])
                    # pairwise adds: rp[k] = r[2k] + r[2k+1]
                    rp = rpp.tile([128, CH // 2], FP, tag="rp")
                    p1eng = nc.vector if c == 3 else nc.gpsimd
                    p1eng.tensor_tensor(
                        rp[:], rr[:, 0::2], rr[:, 1::2], op=OP.add,
                    )
                    ds = dsum[:, c * JC:(c + 1) * JC]
                    nc.gpsimd.tensor_tensor(
                        ds, rp[:, 0::2], rp[:, 1::2], op=OP.add,
                    )
                    if c % 2 == 1:
                        # 1/(d+4) via exp(-ln((d+4)c) - 64 ln2) on ACT
                        ds2 = dsum[:, (c - 1) * JC:(c + 1) * JC]
                        nc.scalar.activation(ds2, ds2, AF.Ln,
                                             bias=bias_4c[:], scale=LNSCALE)
                        nc.scalar.activation(ds2, ds2, AF.Exp,
                                             bias=bias_mln[:], scale=-1.0)
                        for cc in (c - 1, c):
                            rrc = big[:, cc * CH:(cc + 1) * CH]
                            rr3 = rrc.rearrange("p (j m) -> p j m", m=M)
                            dsc = dsum[:, cc * JC:(cc + 1) * JC]
                            nc.vector.scalar_tensor_tensor(
                                rr3, rr3, 1.0,
                                dsc.broadcast_to((128, JC, M)),
                                op0=OP.add, op1=OP.mult,
                            )
                if (ib, h) == (R // 128 - 1, 1):
                    for piece in range(2):
                        p0 = piece * (HALF // 2)
                        nc.sync.dma_start(
                            out[ib * 128:(ib + 1) * 128,
                                f0 + p0:f0 + p0 + HALF // 2],
                            big[:, p0:p0 + HALF // 2],
                        )
                else:
                    nc.sync.dma_start(
                        out[ib * 128:(ib + 1) * 128, f0:f0 + HALF], big[:],
                    )
    ctx.close()


def kernel(f_meta: np.ndarray, W: np.ndarray) -> np.ndarray:
    f_meta = np.ascontiguousarray(f_meta, dtype=np.float32)
    W = np.ascontiguousarray(W, dtype=np.float32)
    assert f_meta.shape == (N, D) and W.shape == (2 * D, M)

    if "nc" not in _CACHE:
        _CACHE["nc"] = _build_nc()
    nc = _CACHE["nc"]

    import ml_dtypes
    mask = np.zeros((3 * M, F), dtype=ml_dtypes.bfloat16)
    for k in range(3):
        for m in range(M):
            mask[k * M + m, m::M] = 1.0
    fT = np.ascontiguousarray(f_meta.T)
    in_maps = [
        {
            "fT_full": fT,
            "fT_mine": np.ascontiguousarray(fT[:, c * R:(c + 1) * R]),
            "s_mask": mask,
            "w": W,
        }
        for c in range(NCORES)
    ]
    res = run_bass_kernel_spmd(nc, in_maps, core_ids=list(range(NCORES)))
    slabs = [res.results[c]["out"] for c in range(NCORES)]
    return np.concatenate(slabs, axis=0).reshape(N, N, M)


if __name__ == "__main__":
    f = np.random.randn(N, D).astype(np.float32)
    w = np.random.randn(2 * D, M).astype(np.float32)
    o = kernel(f, w)
    print(o.shape, o.dtype, o[0, 0], o.sum(axis=-1).mean())
